# revision 14
# baseline (speedup 1.0000x reference)
"""Trainium2 Bass kernel for DangoPreTrain-style GNN pretraining forward.

Model (per edge type t of 3):
    h1 = relu(SAGE(emb, edges_t; W1l, b1, W1r))
    h2 = relu(SAGE(h1,  edges_t; W2l, b2, W2r))
    recon = h2 @ Wrec_t.T + brec_t            # [N, N]
Returns (embeddings [3,N,64], reconstructions [3,N,N], x_init [N,64]).

Strategy:
  The sparse mean-aggregation  agg[i] = mean_{(j->i) in E} x[j]  is
  reformulated as a dense GEMM with the (transposed) edge-count matrix:
      CT[src, dst] = #edges (src -> dst)      (counts <= ~4, exact in bf16)
      aggT = x^T @ CT,  then scale columns by 1/deg.
  CT is built on host from edge_index (bincount), stored bf16 and sharded
  over the 8 NeuronCores by destination-node blocks (832 columns each).
  Each core computes its dst-shard of h1 for all 3 types (launch 1), the
  host re-gathers/transposes h1, then launch 2 computes the dst-shard of
  h2 plus the corresponding 832-row slab of every recon matrix.
  All big GEMMs run in bf16 with fp32 PSUM accumulation; the small 64x64
  linear transforms run in fp32.
"""

import os
import sys

import numpy as np
import ml_dtypes

sys.path.insert(0, "/opt/trn_rl_repo")

import concourse.bass as bass
import concourse.bacc as bacc
import concourse.mybir as mybir
import concourse.tile as tile
from concourse import bass2jax

BF16 = ml_dtypes.bfloat16
F32 = mybir.dt.float32
BF = mybir.dt.bfloat16
RELU = mybir.ActivationFunctionType.Relu

N = 6607      # nodes
H = 64        # hidden
T = 3         # edge types
NP = 6656     # N padded to 52*128
KCH = NP // 128   # 52 contraction chunks
NC = 8        # cores
S = NP // NC  # 832 dst columns per core
FB = [(0, 512), (512, 320)]          # free-dim blocks of the 832-wide shard
IB = [(i * 128, 128) for i in range(6)] + [(768, 64)]  # recon row blocks
NB = [(i * 512, 512) for i in range(13)]               # recon col blocks

# exec time (ns) of the last kernel() call's device launches, when timed
# (set BASS_TIME_ITERS=<n> to measure marginal per-iteration device time)
LAST_EXEC_NS = None
LAST_NS_PARTS = None


def _run_spmd(nc, in_maps):
    """Run an SPMD Bass program on the 8 cores via PJRT (axon).

    Mirrors concourse.bass2jax.run_bass_via_pjrt's multi-core path, but
    keeps inputs resident on device so optional timing iterations measure
    device execution rather than host->device transfer. Returns
    (results_per_core, per_iter_ns_or_None).
    """
    import jax
    import jax.numpy as jnp
    from jax.sharding import Mesh, PartitionSpec, NamedSharding
    from jax.experimental.shard_map import shard_map
    import concourse.mybir as mybir_

    bass2jax.install_neuronx_cc_hook()
    n_cores = len(in_maps)
    partition_name = (nc.partition_id_tensor.name
                      if nc.partition_id_tensor else None)

    in_names, out_names, out_avals, zero_shapes = [], [], [], []
    for alloc in nc.m.functions[0].allocations:
        if not isinstance(alloc, mybir_.MemoryLocationSet):
            continue
        name = alloc.memorylocations[0].name
        if alloc.kind == "ExternalInput":
            if name != partition_name:
                in_names.append(name)
        elif alloc.kind == "ExternalOutput":
            shape = tuple(alloc.tensor_shape)
            dtype = mybir_.dt.np(alloc.dtype)
            out_names.append(name)
            out_avals.append(jax.core.ShapedArray(shape, dtype))
            zero_shapes.append((shape, dtype))
    n_params = len(in_names)
    n_outs = len(out_avals)
    all_in_names = list(in_names) + list(out_names)
    if partition_name is not None:
        all_in_names = all_in_names + [partition_name]
    donate = tuple(range(n_params, n_params + n_outs))

    def _body(*args):
        operands = list(args)
        if partition_name is not None:
            operands.append(bass2jax.partition_id_tensor())
        outs = bass2jax._bass_exec_p.bind(
            *operands,
            out_avals=tuple(out_avals),
            in_names=tuple(all_in_names),
            out_names=tuple(out_names),
            lowering_input_output_aliases=(),
            sim_require_finite=True,
            sim_require_nnan=True,
            nc=nc,
        )
        return tuple(outs)

    devices = jax.devices()[:n_cores]
    mesh = Mesh(np.asarray(devices), ("core",))
    spec = NamedSharding(mesh, PartitionSpec("core"))
    in_specs = (PartitionSpec("core"),) * (n_params + n_outs)
    out_specs = (PartitionSpec("core"),) * n_outs
    sharded = jax.jit(
        shard_map(_body, mesh=mesh, in_specs=in_specs, out_specs=out_specs,
                  check_rep=False),
        donate_argnums=donate, keep_unused=True)

    concat_in = [
        jax.device_put(
            np.concatenate([np.asarray(m[name]) for m in in_maps], axis=0),
            spec)
        for name in in_names
    ]

    def _zeros():
        return [
            jax.jit(lambda s=s, d=d: jnp.zeros((n_cores * s[0],) + s[1:], d),
                    out_shardings=spec)()
            for (s, d) in zero_shapes
        ]

    out_arrs = sharded(*concat_in, *_zeros())
    jax.block_until_ready(out_arrs)

    per_iter_ns = None
    iters = int(os.environ.get("BASS_TIME_ITERS", "0"))
    if iters > 0:
        import time
        warm = min(2, iters)
        zsets = [_zeros() for _ in range(iters + warm)]
        jax.block_until_ready(zsets)
        for i in range(warm):
            jax.block_until_ready(sharded(*concat_in, *zsets[i]))
        t0 = time.perf_counter()
        last = None
        for i in range(warm, warm + iters):
            last = sharded(*concat_in, *zsets[i])
        jax.block_until_ready(last)
        per_iter_ns = (time.perf_counter() - t0) / iters * 1e9
        del zsets, last

    results = [
        {name: np.asarray(out_arrs[i]).reshape(
            n_cores, *out_avals[i].shape)[c]
         for i, name in enumerate(out_names)}
        for c in range(n_cores)
    ]
    return results, per_iter_ns


def _layer_gemm(nc, tc, pools, ct_dram, t, xch_tile, xT_tile, rb_tile,
                wl_tile, wr_tile, bias_tile, out_f32_tile):
    """One SAGE layer for edge type t on this core's dst shard.

    aggT = x^T @ CT_shard  (psum, bf16 matmuls over 52 src chunks)
    mean = aggT * rdeg     (DVE, fp32)
    hT   = relu(Wl @ mean + Wr @ xT + b)   (fp32 matmuls + scalar act)
    Writes hT [64, 832] fp32 into out_f32_tile.
    """
    ctp, psum_a, psum_w, work = pools
    psA = psum_a.tile([64, 512], F32)
    psB = psum_a.tile([64, 320], F32)
    for k in range(KCH):
        ctA = ctp.tile([128, 512], BF, tag="ctA")
        ctB = ctp.tile([128, 320], BF, tag="ctB")
        nc.sync.dma_start(ctA[:], ct_dram[t, k * 128:(k + 1) * 128, 0:512])
        nc.sync.dma_start(ctB[:], ct_dram[t, k * 128:(k + 1) * 128, 512:832])
        lhs = xch_tile[:, k * H:(k + 1) * H]
        nc.tensor.matmul(psA[:], lhs, ctA[:],
                         start=(k == 0), stop=(k == KCH - 1))
        nc.tensor.matmul(psB[:], lhs, ctB[:],
                         start=(k == 0), stop=(k == KCH - 1))
    mm = work.tile([64, S], F32)
    nc.vector.tensor_mul(mm[:, 0:512], psA[:], rb_tile[:, 0:512])
    nc.vector.tensor_mul(mm[:, 512:832], psB[:], rb_tile[:, 512:832])
    for (f0, fs) in FB:
        psW = psum_w.tile([64, 512], F32)
        nc.tensor.matmul(psW[:, :fs], wl_tile[:], mm[:, f0:f0 + fs],
                         start=True, stop=False)
        nc.tensor.matmul(psW[:, :fs], wr_tile[:], xT_tile[:, f0:f0 + fs],
                         start=False, stop=True)
        nc.scalar.activation(out_f32_tile[:, f0:f0 + fs], psW[:, :fs],
                             RELU, bias=bias_tile[:])


def _build_layer1():
    nc = bacc.Bacc("TRN2", target_bir_lowering=False, debug=False,
                   num_devices=NC, num_swdge_queues=4)
    ct = nc.dram_tensor("ct", [T, NP, S], BF, kind="ExternalInput")
    xch = nc.dram_tensor("xch", [128, KCH * H], BF, kind="ExternalInput")
    xT = nc.dram_tensor("xT", [H, S], F32, kind="ExternalInput")
    rb = nc.dram_tensor("rb", [T, H, S], F32, kind="ExternalInput")
    wl = nc.dram_tensor("wl", [T, H, H], F32, kind="ExternalInput")
    wr = nc.dram_tensor("wr", [T, H, H], F32, kind="ExternalInput")
    bias = nc.dram_tensor("bias", [T, H, 1], F32, kind="ExternalInput")
    h1T = nc.dram_tensor("h1T", [T, H, S], F32, kind="ExternalOutput")

    with tile.TileContext(nc) as tc:
        with (
            tc.tile_pool(name="const", bufs=1) as const,
            tc.tile_pool(name="ctp", bufs=3) as ctp,
            tc.tile_pool(name="psum_a", bufs=2, space="PSUM") as psum_a,
            tc.tile_pool(name="psum_w", bufs=2, space="PSUM") as psum_w,
            tc.tile_pool(name="work", bufs=2) as work,
            tc.tile_pool(name="wt", bufs=2) as wt,
            tc.tile_pool(name="outp", bufs=2) as outp,
        ):
            xch_tile = const.tile([128, KCH * H], BF)
            nc.sync.dma_start(xch_tile[:], xch[:])
            xT_tile = const.tile([H, S], F32)
            nc.sync.dma_start(xT_tile[:], xT[:])
            for t in range(T):
                rb_tile = wt.tile([H, S], F32)
                nc.sync.dma_start(rb_tile[:], rb[t])
                wl_tile = wt.tile([H, H], F32)
                nc.sync.dma_start(wl_tile[:], wl[t])
                wr_tile = wt.tile([H, H], F32)
                nc.sync.dma_start(wr_tile[:], wr[t])
                bias_tile = wt.tile([H, 1], F32)
                nc.sync.dma_start(bias_tile[:], bias[t])
                hf = outp.tile([H, S], F32)
                _layer_gemm(nc, tc, (ctp, psum_a, psum_w, work), ct, t,
                            xch_tile, xT_tile, rb_tile, wl_tile, wr_tile,
                            bias_tile, hf)
                nc.sync.dma_start(h1T[t], hf[:])
    nc.compile()
    return nc


def _build_layer2():
    nc = bacc.Bacc("TRN2", target_bir_lowering=False, debug=False,
                   num_devices=NC, num_swdge_queues=4)
    ct = nc.dram_tensor("ct", [T, NP, S], BF, kind="ExternalInput")
    h1ch = nc.dram_tensor("h1ch", [T, 128, KCH * H], BF, kind="ExternalInput")
    h1T = nc.dram_tensor("h1T", [T, H, S], F32, kind="ExternalInput")
    rb = nc.dram_tensor("rb", [T, H, S], F32, kind="ExternalInput")
    wl = nc.dram_tensor("wl", [T, H, H], F32, kind="ExternalInput")
    wr = nc.dram_tensor("wr", [T, H, H], F32, kind="ExternalInput")
    bias = nc.dram_tensor("bias", [T, H, 1], F32, kind="ExternalInput")
    wrp = nc.dram_tensor("wrp", [T, H + 1, NP], BF, kind="ExternalInput")
    h2T = nc.dram_tensor("h2T", [T, H, S], F32, kind="ExternalOutput")
    recon = nc.dram_tensor("recon", [T, S, NP], F32, kind="ExternalOutput")

    with tile.TileContext(nc) as tc:
        with (
            tc.tile_pool(name="xin", bufs=2) as xin,
            tc.tile_pool(name="ctp", bufs=3) as ctp,
            tc.tile_pool(name="psum_a", bufs=1, space="PSUM") as psum_a,
            tc.tile_pool(name="psum_w", bufs=1, space="PSUM") as psum_w,
            tc.tile_pool(name="psum_r", bufs=4, space="PSUM") as psum_r,
            tc.tile_pool(name="work", bufs=2) as work,
            tc.tile_pool(name="wt", bufs=2) as wt,
            tc.tile_pool(name="outp", bufs=2) as outp,
        ):
            for t in range(T):
                xch_tile = xin.tile([128, KCH * H], BF)
                nc.sync.dma_start(xch_tile[:], h1ch[t])
                xT_tile = xin.tile([H, S], F32)
                nc.sync.dma_start(xT_tile[:], h1T[t])
                wrp_tile = xin.tile([H + 1, NP], BF)
                nc.sync.dma_start(wrp_tile[:], wrp[t])
                rb_tile = wt.tile([H, S], F32)
                nc.sync.dma_start(rb_tile[:], rb[t])
                wl_tile = wt.tile([H, H], F32)
                nc.sync.dma_start(wl_tile[:], wl[t])
                wr_tile = wt.tile([H, H], F32)
                nc.sync.dma_start(wr_tile[:], wr[t])
                bias_tile = wt.tile([H, 1], F32)
                nc.sync.dma_start(bias_tile[:], bias[t])

                hf = outp.tile([H, S], F32)
                _layer_gemm(nc, tc, (ctp, psum_a, psum_w, work), ct, t,
                            xch_tile, xT_tile, rb_tile, wl_tile, wr_tile,
                            bias_tile, hf)
                nc.sync.dma_start(h2T[t], hf[:])

                # bf16 copy of h2T with a trailing ones-row so the recon
                # matmul's 65th contraction element adds brec.
                h2b = work.tile([H + 1, S], BF)
                nc.vector.tensor_copy(h2b[0:H, :], hf[:])
                nc.vector.memset(h2b[H:H + 1, :], 1.0)

                for (i0, isz) in IB:
                    stripe = outp.tile([128, NP], F32, tag="stripe")
                    for j, (n0, nsz) in enumerate(NB):
                        psR = psum_r.tile([128, 512], F32)
                        nc.tensor.matmul(psR[:isz, :], h2b[:, i0:i0 + isz],
                                         wrp_tile[:, n0:n0 + nsz],
                                         start=True, stop=True)
                        # PSUM can't be DMA'd; drain to SBUF on DVE/ACT
                        if j % 2 == 0:
                            nc.vector.tensor_copy(
                                stripe[:isz, n0:n0 + nsz], psR[:isz, :])
                        else:
                            nc.scalar.copy(
                                stripe[:isz, n0:n0 + nsz], psR[:isz, :])
                    nc.sync.dma_start(recon[t, i0:i0 + isz, :],
                                      stripe[:isz, :])
    nc.compile()
    return nc


def _chunked(x):
    """[NP, H] row-major -> [128, KCH*H] where out[p, k*H:h] = x[k*128+p, h]."""
    return np.ascontiguousarray(
        x.reshape(KCH, 128, H).transpose(1, 0, 2).reshape(128, KCH * H))


def kernel(emb, edge_index, W1l, b1, W1r, W2l, b2, W2r, Wrec, brec):
    global LAST_EXEC_NS
    LAST_EXEC_NS = None

    emb = np.asarray(emb, dtype=np.float32)
    ei = np.asarray(edge_index)
    W1l, b1, W1r = (np.asarray(a, np.float32) for a in (W1l, b1, W1r))
    W2l, b2, W2r = (np.asarray(a, np.float32) for a in (W2l, b2, W2r))
    Wrec, brec = np.asarray(Wrec, np.float32), np.asarray(brec, np.float32)

    # ---- host prep: edge-count matrices, degrees, padded/transposed views
    ct_shards = np.zeros((NC, T, NP, S), dtype=BF16)
    rdeg = np.zeros((T, NP), dtype=np.float32)
    for t in range(T):
        src = ei[t, 0].astype(np.int64)
        dst = ei[t, 1].astype(np.int64)
        cnt = np.bincount(src * NP + dst, minlength=N * NP).reshape(N, NP)
        deg = np.bincount(dst, minlength=N)
        rdeg[t, :N] = 1.0 / np.maximum(deg, 1.0)
        cnt = cnt.astype(np.float32)
        for c in range(NC):
            ct_shards[c, t, :N, :] = cnt[:, c * S:(c + 1) * S]

    embp = np.zeros((NP, H), dtype=np.float32)
    embp[:N] = emb
    xch_bf = _chunked(embp).astype(BF16)
    embT = np.ascontiguousarray(embp.T)                      # [64, NP] f32
    rb = np.ascontiguousarray(
        np.broadcast_to(rdeg[:, None, :], (T, H, NP)))       # [T, 64, NP]
    w1lT = np.ascontiguousarray(W1l.transpose(0, 2, 1))
    w1rT = np.ascontiguousarray(W1r.transpose(0, 2, 1))
    w2lT = np.ascontiguousarray(W2l.transpose(0, 2, 1))
    w2rT = np.ascontiguousarray(W2r.transpose(0, 2, 1))
    b1c = np.ascontiguousarray(b1[:, :, None])
    b2c = np.ascontiguousarray(b2[:, :, None])
    wrp = np.zeros((T, H + 1, NP), dtype=BF16)
    for t in range(T):
        wrp[t, :H, :N] = Wrec[t].T
        wrp[t, H, :N] = brec[t]

    core_ids = list(range(NC))

    # ---- launch 1: layer 1, dst-sharded over 8 cores
    nc1 = _build_layer1()
    in_maps1 = [
        dict(ct=ct_shards[c], xch=xch_bf,
             xT=np.ascontiguousarray(embT[:, c * S:(c + 1) * S]),
             rb=np.ascontiguousarray(rb[:, :, c * S:(c + 1) * S]),
             wl=w1lT, wr=w1rT, bias=b1c)
        for c in core_ids
    ]
    res1, ns1 = _run_spmd(nc1, in_maps1)

    h1T = np.concatenate([res1[c]["h1T"] for c in core_ids], axis=2)
    h1 = np.ascontiguousarray(h1T.transpose(0, 2, 1))        # [T, NP, 64]
    h1[:, N:, :] = 0.0
    h1ch = np.stack([_chunked(h1[t]) for t in range(T)]).astype(BF16)

    # ---- launch 2: layer 2 + recon slab, dst-sharded over 8 cores
    nc2 = _build_layer2()
    in_maps2 = [
        dict(ct=ct_shards[c], h1ch=h1ch,
             h1T=np.ascontiguousarray(h1T[:, :, c * S:(c + 1) * S]),
             rb=np.ascontiguousarray(rb[:, :, c * S:(c + 1) * S]),
             wl=w2lT, wr=w2rT, bias=b2c, wrp=wrp)
        for c in core_ids
    ]
    res2, ns2 = _run_spmd(nc2, in_maps2)

    h2T = np.concatenate([res2[c]["h2T"] for c in core_ids], axis=2)
    embeddings = np.ascontiguousarray(h2T.transpose(0, 2, 1)[:, :N, :])
    reconstructions = np.empty((T, N, N), dtype=np.float32)
    for c in core_ids:
        r0 = c * S
        r1 = min(r0 + S, N)
        if r1 > r0:
            reconstructions[:, r0:r1, :] = \
                res2[c]["recon"][:, :r1 - r0, :N]

    if ns1 is not None and ns2 is not None:
        LAST_EXEC_NS = int(ns1 + ns2)
        global LAST_NS_PARTS
        LAST_NS_PARTS = (ns1, ns2)
    return embeddings, reconstructions, emb


# revision 32
# speedup vs baseline: 49.8275x; 49.8275x over previous
"""Trainium2 Bass kernel for DangoPreTrain-style GNN pretraining forward.

Model (per edge type t of 3):
    h1 = relu(SAGE(emb, edges_t; W1l, b1, W1r))
    h2 = relu(SAGE(h1,  edges_t; W2l, b2, W2r))
    recon = h2 @ Wrec_t.T + brec_t            # [N, N]
Returns (embeddings [3,N,64], reconstructions [3,N,N], x_init [N,64]).

Strategy:
  The sparse mean-aggregation  agg[i] = mean_{(j->i) in E} x[j]  is
  reformulated as a dense GEMM with the (transposed) edge-count matrix:
      CT[src, dst] = #edges (src -> dst)      (counts <= ~4, exact in bf16)
      aggT = x^T @ CT,  then scale columns by 1/deg.
  CT is built on host from edge_index (bincount), stored bf16 and sharded
  over the 8 NeuronCores by destination-node blocks (832 columns each).
  Each core computes its dst-shard of h1 for all 3 types (launch 1), the
  host re-gathers/transposes h1, then launch 2 computes the dst-shard of
  h2 plus the corresponding 832-row slab of every recon matrix.
  All big GEMMs run in bf16 with fp32 PSUM accumulation; the small 64x64
  linear transforms run in fp32.
"""

import os
import sys

import numpy as np
import ml_dtypes

sys.path.insert(0, "/opt/trn_rl_repo")

import concourse.bass as bass
import concourse.bacc as bacc
import concourse.mybir as mybir
import concourse.tile as tile
from concourse import bass2jax
from concourse.bass_utils import run_bass_kernel_spmd


def _ensure_ntff_hook():
    """Register the NTFF profile hook missing from this image's antenv.

    trn_boot tries to install it but antenv lacks the axon_hooks module;
    shim it in-process so run_bass_kernel_spmd(trace=True) can profile.
    """
    try:
        from antenv import axon_hooks  # noqa: F401
        return True
    except ImportError:
        pass
    try:
        import types
        import antenv
        if "/root/.axon_site" not in sys.path:
            sys.path.insert(0, "/root/.axon_site")
        from trn_agent_boot.trn_boot import _ntff_profile_via_ctypes
        hook = _ntff_profile_via_ctypes("/opt/axon/libaxon_pjrt.so")
        if hook is None:
            return False
        m = types.ModuleType("antenv.axon_hooks")
        m.get_axon_ntff_profile_hook = lambda: hook
        m.set_axon_ntff_profile_hook = lambda h: None
        sys.modules["antenv.axon_hooks"] = m
        antenv.axon_hooks = m
        return True
    except Exception:
        return False

BF16 = ml_dtypes.bfloat16
F32 = mybir.dt.float32
BF = mybir.dt.bfloat16
RELU = mybir.ActivationFunctionType.Relu

N = 6607      # nodes
H = 64        # hidden
T = 3         # edge types
NP = 6656     # N padded to 52*128
KCH = NP // 128   # 52 contraction chunks
NC = 8        # cores
S = NP // NC  # 832 dst columns per core
FB = [(0, 512), (512, 320)]          # free-dim blocks of the 832-wide shard
IB = [(i * 128, 128) for i in range(6)] + [(768, 64)]  # recon row blocks
NB = [(i * 512, 512) for i in range(13)]               # recon col blocks

# exec time (ns) of the last kernel() call's device launches, when timed
# (set BASS_TIME_ITERS=<n> to measure marginal per-iteration device time)
LAST_EXEC_NS = None
LAST_NS_PARTS = None


def _run_spmd(nc, in_maps):
    """Run an SPMD Bass program on the 8 cores via PJRT (axon).

    Mirrors concourse.bass2jax.run_bass_via_pjrt's multi-core path, but
    keeps inputs resident on device so optional timing iterations measure
    device execution rather than host->device transfer. Returns
    (results_per_core, per_iter_ns_or_None).
    """
    import jax
    import jax.numpy as jnp
    from jax.sharding import Mesh, PartitionSpec, NamedSharding
    from jax.experimental.shard_map import shard_map
    import concourse.mybir as mybir_

    bass2jax.install_neuronx_cc_hook()
    n_cores = len(in_maps)
    partition_name = (nc.partition_id_tensor.name
                      if nc.partition_id_tensor else None)

    in_names, out_names, out_avals, zero_shapes = [], [], [], []
    for alloc in nc.m.functions[0].allocations:
        if not isinstance(alloc, mybir_.MemoryLocationSet):
            continue
        name = alloc.memorylocations[0].name
        if alloc.kind == "ExternalInput":
            if name != partition_name:
                in_names.append(name)
        elif alloc.kind == "ExternalOutput":
            shape = tuple(alloc.tensor_shape)
            dtype = mybir_.dt.np(alloc.dtype)
            out_names.append(name)
            out_avals.append(jax.core.ShapedArray(shape, dtype))
            zero_shapes.append((shape, dtype))
    n_params = len(in_names)
    n_outs = len(out_avals)
    all_in_names = list(in_names) + list(out_names)
    if partition_name is not None:
        all_in_names = all_in_names + [partition_name]
    donate = tuple(range(n_params, n_params + n_outs))

    def _body(*args):
        operands = list(args)
        if partition_name is not None:
            operands.append(bass2jax.partition_id_tensor())
        outs = bass2jax._bass_exec_p.bind(
            *operands,
            out_avals=tuple(out_avals),
            in_names=tuple(all_in_names),
            out_names=tuple(out_names),
            lowering_input_output_aliases=(),
            sim_require_finite=True,
            sim_require_nnan=True,
            nc=nc,
        )
        return tuple(outs)

    devices = jax.devices()[:n_cores]
    mesh = Mesh(np.asarray(devices), ("core",))
    spec = NamedSharding(mesh, PartitionSpec("core"))
    in_specs = (PartitionSpec("core"),) * (n_params + n_outs)
    out_specs = (PartitionSpec("core"),) * n_outs
    sharded = jax.jit(
        shard_map(_body, mesh=mesh, in_specs=in_specs, out_specs=out_specs,
                  check_rep=False),
        donate_argnums=donate, keep_unused=True)

    concat_in = [
        jax.device_put(
            np.concatenate([np.asarray(m[name]) for m in in_maps], axis=0),
            spec)
        for name in in_names
    ]

    def _zeros():
        return [
            jax.jit(lambda s=s, d=d: jnp.zeros((n_cores * s[0],) + s[1:], d),
                    out_shardings=spec)()
            for (s, d) in zero_shapes
        ]

    out_arrs = sharded(*concat_in, *_zeros())
    jax.block_until_ready(out_arrs)

    per_iter_ns = None

    results = [
        {name: np.asarray(out_arrs[i]).reshape(
            n_cores, *out_avals[i].shape)[c]
         for i, name in enumerate(out_names)}
        for c in range(n_cores)
    ]
    return results, per_iter_ns


def _layer_gemm(nc, tc, pools, ct_dram, t, xch_tile, xT_tile, rb_tile,
                wl_tile, wr_tile, bias_tile, out_f32_tile):
    """One SAGE layer for edge type t on this core's dst shard.

    aggT = x^T @ CT_shard  (psum, bf16 matmuls over 52 src chunks)
    mean = aggT * rdeg     (DVE, fp32)
    hT   = relu(Wl @ mean + Wr @ xT + b)   (fp32 matmuls + scalar act)
    Writes hT [64, 832] fp32 into out_f32_tile.
    """
    ctp, psum_a, psum_w, work = pools
    psA = psum_a.tile([64, 512], F32)
    psB = psum_a.tile([64, 320], F32)
    for k in range(KCH):
        ctt = ctp.tile([128, S], BF)
        nc.sync.dma_start(ctt[:], ct_dram[t, k * 128:(k + 1) * 128, :])
        lhs = xch_tile[:, k * H:(k + 1) * H]
        nc.tensor.matmul(psA[:], lhs, ctt[:, 0:512],
                         start=(k == 0), stop=(k == KCH - 1))
        nc.tensor.matmul(psB[:], lhs, ctt[:, 512:832],
                         start=(k == 0), stop=(k == KCH - 1))
    mm = work.tile([64, S], F32)
    nc.vector.tensor_mul(mm[:, 0:512], psA[:], rb_tile[:, 0:512])
    nc.vector.tensor_mul(mm[:, 512:832], psB[:], rb_tile[:, 512:832])
    for (f0, fs) in FB:
        psW = psum_w.tile([64, 512], F32)
        nc.tensor.matmul(psW[:, :fs], wl_tile[:], mm[:, f0:f0 + fs],
                         start=True, stop=False)
        nc.tensor.matmul(psW[:, :fs], wr_tile[:], xT_tile[:, f0:f0 + fs],
                         start=False, stop=True)
        nc.scalar.activation(out_f32_tile[:, f0:f0 + fs], psW[:, :fs],
                             RELU, bias=bias_tile[:])


def _build_layer1():
    nc = bacc.Bacc("TRN2", target_bir_lowering=False, debug=False,
                   num_devices=NC, num_swdge_queues=4)
    ct = nc.dram_tensor("ct", [T, NP, S], BF, kind="ExternalInput")
    xch = nc.dram_tensor("xch", [128, KCH * H], BF, kind="ExternalInput")
    xT = nc.dram_tensor("xT", [H, S], F32, kind="ExternalInput")
    rb = nc.dram_tensor("rb", [T, H, S], F32, kind="ExternalInput")
    wl = nc.dram_tensor("wl", [T, H, H], F32, kind="ExternalInput")
    wr = nc.dram_tensor("wr", [T, H, H], F32, kind="ExternalInput")
    bias = nc.dram_tensor("bias", [T, H, 1], F32, kind="ExternalInput")
    h1T = nc.dram_tensor("h1T", [T, H, S], F32, kind="ExternalOutput")

    with tile.TileContext(nc) as tc:
        with (
            tc.tile_pool(name="const", bufs=1) as const,
            tc.tile_pool(name="ctp", bufs=3) as ctp,
            tc.tile_pool(name="psum_a", bufs=2, space="PSUM") as psum_a,
            tc.tile_pool(name="psum_w", bufs=2, space="PSUM") as psum_w,
            tc.tile_pool(name="work", bufs=2) as work,
            tc.tile_pool(name="wt", bufs=2) as wt,
            tc.tile_pool(name="outp", bufs=2) as outp,
        ):
            xch_tile = const.tile([128, KCH * H], BF)
            nc.sync.dma_start(xch_tile[:], xch[:])
            xT_tile = const.tile([H, S], F32R)
            nc.sync.dma_start(xT_tile[:], xT[:])
            for t in range(T):
                rb_tile = wt.tile([H, S], F32)
                nc.sync.dma_start(rb_tile[:], rb[t])
                wl_tile = wt.tile([H, H], F32)
                nc.sync.dma_start(wl_tile[:], wl[t])
                wr_tile = wt.tile([H, H], F32)
                nc.sync.dma_start(wr_tile[:], wr[t])
                bias_tile = wt.tile([H, 1], F32)
                nc.sync.dma_start(bias_tile[:], bias[t])
                hf = outp.tile([H, S], F32)
                _layer_gemm(nc, tc, (ctp, psum_a, psum_w, work), ct, t,
                            xch_tile, xT_tile, rb_tile, wl_tile, wr_tile,
                            bias_tile, hf)
                nc.sync.dma_start(h1T[t], hf[:])
    nc.compile()
    return nc


def _build_layer2():
    nc = bacc.Bacc("TRN2", target_bir_lowering=False, debug=False,
                   num_devices=NC, num_swdge_queues=4)
    ct = nc.dram_tensor("ct", [T, NP, S], BF, kind="ExternalInput")
    h1ch = nc.dram_tensor("h1ch", [T, 128, KCH * H], BF, kind="ExternalInput")
    h1T = nc.dram_tensor("h1T", [T, H, S], F32, kind="ExternalInput")
    rb = nc.dram_tensor("rb", [T, H, S], F32, kind="ExternalInput")
    wl = nc.dram_tensor("wl", [T, H, H], F32, kind="ExternalInput")
    wr = nc.dram_tensor("wr", [T, H, H], F32, kind="ExternalInput")
    bias = nc.dram_tensor("bias", [T, H, 1], F32, kind="ExternalInput")
    wrp = nc.dram_tensor("wrp", [T, H + 1, NP], BF, kind="ExternalInput")
    h2T = nc.dram_tensor("h2T", [T, H, S], F32, kind="ExternalOutput")
    recon = nc.dram_tensor("recon", [T, S, NP], F32, kind="ExternalOutput")

    with tile.TileContext(nc) as tc:
        with (
            tc.tile_pool(name="xin", bufs=2) as xin,
            tc.tile_pool(name="ctp", bufs=3) as ctp,
            tc.tile_pool(name="psum_a", bufs=1, space="PSUM") as psum_a,
            tc.tile_pool(name="psum_w", bufs=1, space="PSUM") as psum_w,
            tc.tile_pool(name="psum_r", bufs=4, space="PSUM") as psum_r,
            tc.tile_pool(name="work", bufs=2) as work,
            tc.tile_pool(name="wt", bufs=2) as wt,
            tc.tile_pool(name="outp", bufs=2) as outp,
        ):
            for t in range(T):
                xch_tile = xin.tile([128, KCH * H], BF)
                nc.sync.dma_start(xch_tile[:], h1ch[t])
                xT_tile = xin.tile([H, S], F32)
                nc.sync.dma_start(xT_tile[:], h1T[t])
                wrp_tile = xin.tile([H + 1, NP], BF)
                nc.sync.dma_start(wrp_tile[:], wrp[t])
                rb_tile = wt.tile([H, S], F32)
                nc.sync.dma_start(rb_tile[:], rb[t])
                wl_tile = wt.tile([H, H], F32)
                nc.sync.dma_start(wl_tile[:], wl[t])
                wr_tile = wt.tile([H, H], F32)
                nc.sync.dma_start(wr_tile[:], wr[t])
                bias_tile = wt.tile([H, 1], F32)
                nc.sync.dma_start(bias_tile[:], bias[t])

                hf = outp.tile([H, S], F32)
                _layer_gemm(nc, tc, (ctp, psum_a, psum_w, work), ct, t,
                            xch_tile, xT_tile, rb_tile, wl_tile, wr_tile,
                            bias_tile, hf)
                nc.sync.dma_start(h2T[t], hf[:])

                # bf16 copy of h2T with a trailing ones-row so the recon
                # matmul's 65th contraction element adds brec.
                h2b = work.tile([H + 1, S], BF)
                nc.vector.tensor_copy(h2b[0:H, :], hf[:])
                nc.vector.memset(h2b[H:H + 1, :], 1.0)

                for (i0, isz) in IB:
                    stripe = outp.tile([128, NP], F32, tag="stripe")
                    for j, (n0, nsz) in enumerate(NB):
                        psR = psum_r.tile([128, 512], F32)
                        nc.tensor.matmul(psR[:isz, :], h2b[:, i0:i0 + isz],
                                         wrp_tile[:, n0:n0 + nsz],
                                         start=True, stop=True)
                        # PSUM can't be DMA'd; drain to SBUF on DVE/ACT
                        if j % 2 == 0:
                            nc.vector.tensor_copy(
                                stripe[:isz, n0:n0 + nsz], psR[:isz, :])
                        else:
                            nc.scalar.copy(
                                stripe[:isz, n0:n0 + nsz], psR[:isz, :])
                    nc.sync.dma_start(recon[t, i0:i0 + isz, :],
                                      stripe[:isz, :])
    nc.compile()
    return nc


def _build_merged():
    """Single-launch kernel, software-pipelined one type deep:
    L1(t) -> AllGather(t) overlaps L1(t+1); L2(t)/recon(t) run after.
    CT shard loaded once per type (fp8_e4m3, counts exact), reused by
    both layers from SBUF."""
    from concourse.masks import make_identity
    FP8 = mybir.dt.float8e4
    F32R = mybir.dt.float32r

    nc = bacc.Bacc("TRN2", target_bir_lowering=False, debug=False,
                   num_devices=NC)
    ctc = nc.dram_tensor("ctc", [T, 128, KCH * S], FP8, kind="ExternalInput")
    xch = nc.dram_tensor("xch", [128, KCH * H], BF, kind="ExternalInput")
    xT = nc.dram_tensor("xT", [H, S], F32R, kind="ExternalInput")
    rb = nc.dram_tensor("rb", [T, H, S], F32, kind="ExternalInput")
    w1l = nc.dram_tensor("w1l", [T, H, H], F32R, kind="ExternalInput")
    w1r = nc.dram_tensor("w1r", [T, H, H], F32R, kind="ExternalInput")
    b1 = nc.dram_tensor("b1", [T, H, 1], F32, kind="ExternalInput")
    w2l = nc.dram_tensor("w2l", [T, H, H], F32R, kind="ExternalInput")
    w2r = nc.dram_tensor("w2r", [T, H, H], F32R, kind="ExternalInput")
    b2 = nc.dram_tensor("b2", [T, H, 1], F32, kind="ExternalInput")
    wrp = nc.dram_tensor("wrp", [T, H + 1, NP], BF, kind="ExternalInput")
    h2T = nc.dram_tensor("h2T", [T, H, S], F32, kind="ExternalOutput")
    recon = nc.dram_tensor("recon", [T, S, NP], F32, kind="ExternalOutput")

    NBA = NB[:7]
    CA = 7 * 512
    CB = NP - CA

    with tile.TileContext(nc) as tc:
        with (
            tc.tile_pool(name="const", bufs=1) as const,
            tc.tile_pool(name="ctp", bufs=2) as ctp,
            tc.tile_pool(name="hin", bufs=1) as hin,
            tc.tile_pool(name="wrpp", bufs=2) as wrpp,
            tc.tile_pool(name="wt", bufs=2) as wt,
            tc.tile_pool(name="work", bufs=2) as work,
            tc.tile_pool(name="outp", bufs=2) as outp,
            tc.tile_pool(name="psum_a", bufs=1, space="PSUM") as psum_a,
            tc.tile_pool(name="psum_w", bufs=1, space="PSUM") as psum_w,
            tc.tile_pool(name="psum_t", bufs=1, space="PSUM") as psum_t,
            tc.tile_pool(name="psum_r", bufs=4, space="PSUM") as psum_r,
            tc.tile_pool(name="dram", bufs=3, space="DRAM") as dram,
        ):
            xch_tile = const.tile([128, KCH * H], BF)
            nc.sync.dma_start(xch_tile[:], xch[:])
            xT_tile = const.tile([H, S], F32R)
            nc.sync.dma_start(xT_tile[:], xT[:])
            ident = const.tile([H, H], BF)
            make_identity(nc, ident)

            def loads(t):
                ct_big = ctp.tile([128, KCH * S], FP8, tag="ct")
                nc.sync.dma_start(ct_big[:], ctc[t])
                wrp_tile = wrpp.tile([H + 1, NP], BF, tag="wrp")
                nc.sync.dma_start(wrp_tile[:], wrp[t])
                tl = dict(ct=ct_big, wrp=wrp_tile)
                for nm, src_ in (("rb", rb), ("w1l", w1l), ("w1r", w1r),
                                 ("w2l", w2l), ("w2r", w2r)):
                    dt_ = F32 if nm == "rb" else F32R
                    shp = [H, S] if nm == "rb" else [H, H]
                    tile_ = wt.tile(shp, dt_, tag=nm, name=nm + "_t")
                    nc.sync.dma_start(tile_[:], src_[t])
                    tl[nm] = tile_
                for nm, src_ in (("b1", b1), ("b2", b2)):
                    tile_ = wt.tile([H, 1], F32, tag=nm, name=nm + "_t")
                    nc.sync.dma_start(tile_[:], src_[t])
                    tl[nm] = tile_
                return tl

            def agg(lhs_tile, ct_big, rb_tile, wl, wr, xTr, bcol, out):
                psA = psum_a.tile([64, 512], F32, tag="psA", name="psA")
                psB = psum_a.tile([64, 320], F32, tag="psB", name="psB")
                for k in range(KCH):
                    lhs = lhs_tile[:, k * H:(k + 1) * H]
                    nc.tensor.matmul(psA[:], lhs,
                                     ct_big[:, k * S:k * S + 512],
                                     start=(k == 0), stop=(k == KCH - 1))
                    nc.tensor.matmul(psB[:], lhs,
                                     ct_big[:, k * S + 512:(k + 1) * S],
                                     start=(k == 0), stop=(k == KCH - 1))
                mm = work.tile([64, S], F32R, tag="mm", name="mm", bufs=1)
                nc.vector.tensor_mul(mm[:, 0:512], psA[:], rb_tile[:, 0:512])
                nc.vector.tensor_mul(mm[:, 512:S], psB[:], rb_tile[:, 512:S])
                for (f0, fs) in FB:
                    psW = psum_w.tile([64, 512], F32, name="psW")
                    nc.tensor.matmul(psW[:, :fs], wl[:], mm[:, f0:f0 + fs],
                                     start=True, stop=False)
                    nc.tensor.matmul(psW[:, :fs], wr[:], xTr[:, f0:f0 + fs],
                                     start=False, stop=True)
                    nc.scalar.activation(out[:, f0:f0 + fs], psW[:, :fs],
                                         RELU, bias=bcol[:])

            def do_l1(t, tl):
                h1T_own = work.tile([H, S], F32R, tag="h1T_own")
                agg(xch_tile, tl["ct"], tl["rb"], tl["w1l"], tl["w1r"],
                    xT_tile, tl["b1"], h1T_own)
                h1Tb = work.tile([H, S], BF, tag="h1Tb", bufs=1)
                nc.vector.tensor_copy(h1Tb[:], h1T_own[:])
                gin = dram.tile([S, H], BF, tag="gin")
                gout = dram.tile([NP, H], BF, tag="gout",
                                 addr_space="Shared")
                for (j0, jw) in IB:
                    psT = psum_t.tile([128, H], BF, name="psT")
                    nc.tensor.transpose(psT[:jw, :], h1Tb[:, j0:j0 + jw],
                                        ident[:])
                    trs = work.tile([128, H], BF, tag="trs", name="trs")
                    nc.vector.tensor_copy(trs[:jw, :], psT[:jw, :])
                    nc.sync.dma_start(gin[j0:j0 + jw, :], trs[:jw, :])
                nc.gpsimd.collective_compute(
                    "AllGather", mybir.AluOpType.bypass,
                    replica_groups=[list(range(NC))],
                    ins=[gin[:]], outs=[gout[:]],
                )
                return h1T_own, gout

            def do_l2(t, tl, h1T_own, gout):
                h1ch = hin.tile([128, KCH * H], BF)
                for k in range(KCH):
                    nc.sync.dma_start(h1ch[:, k * H:(k + 1) * H],
                                      gout[k * 128:(k + 1) * 128, :])
                hf = outp.tile([H, S], F32, tag="hf", bufs=1)
                agg(h1ch, tl["ct"], tl["rb"], tl["w2l"], tl["w2r"],
                    h1T_own, tl["b2"], hf)
                nc.sync.dma_start(h2T[t], hf[:])
                h2b = work.tile([H + 1, S], BF, tag="h2b", bufs=1)
                nc.vector.tensor_copy(h2b[0:H, :], hf[:])
                nc.vector.memset(h2b[H:H + 1, :], 1.0)
                return h2b

            def emit_recon(t, h2b, wrp_tile):
                for (i0, isz) in IB:
                    stA = outp.tile([128, CA], F32, tag="stA", name="stA")
                    stB = outp.tile([128, CB], F32, tag="stB", name="stB")
                    for j, (n0, nsz) in enumerate(NB):
                        psR = psum_r.tile([128, 512], F32, name="psR")
                        nc.tensor.matmul(psR[:isz, :], h2b[:, i0:i0 + isz],
                                         wrp_tile[:, n0:n0 + nsz],
                                         start=True, stop=True)
                        st = stA if n0 < CA else stB
                        o0 = n0 if n0 < CA else n0 - CA
                        if j % 2 == 0:
                            nc.vector.tensor_copy(
                                st[:isz, o0:o0 + nsz], psR[:isz, :])
                        else:
                            nc.scalar.copy(
                                st[:isz, o0:o0 + nsz], psR[:isz, :])
                    nc.sync.dma_start(recon[t, i0:i0 + isz, 0:CA],
                                      stA[:isz, :])
                    nc.sync.dma_start(recon[t, i0:i0 + isz, CA:NP],
                                      stB[:isz, :])

            pending = None   # (t, tl, h1T_own, gout) awaiting L2+recon
            for t in range(T):
                tl = loads(t)
                h1T_own, gout = do_l1(t, tl)
                if pending is not None:
                    pt, ptl, ph1, pgout = pending
                    h2b = do_l2(pt, ptl, ph1, pgout)
                    emit_recon(pt, h2b, ptl["wrp"])
                pending = (t, tl, h1T_own, gout)
            pt, ptl, ph1, pgout = pending
            h2b = do_l2(pt, ptl, ph1, pgout)
            emit_recon(pt, h2b, ptl["wrp"])
    nc.compile()
    return nc


def _chunked(x):
    """[NP, H] row-major -> [128, KCH*H] where out[p, k*H:h] = x[k*128+p, h]."""
    return np.ascontiguousarray(
        x.reshape(KCH, 128, H).transpose(1, 0, 2).reshape(128, KCH * H))


def kernel(emb, edge_index, W1l, b1, W1r, W2l, b2, W2r, Wrec, brec):
    global LAST_EXEC_NS, LAST_NS_PARTS
    LAST_EXEC_NS = None
    LAST_NS_PARTS = None

    emb = np.asarray(emb, dtype=np.float32)
    ei = np.asarray(edge_index)
    W1l, b1, W1r = (np.asarray(a, np.float32) for a in (W1l, b1, W1r))
    W2l, b2, W2r = (np.asarray(a, np.float32) for a in (W2l, b2, W2r))
    Wrec, brec = np.asarray(Wrec, np.float32), np.asarray(brec, np.float32)

    # ---- host prep: edge-count matrices, degrees, padded/transposed views
    merged = not os.environ.get("BASS_TWO_LAUNCH")
    if merged:
        ctc = np.zeros((NC, T, 128, KCH * S), dtype=ml_dtypes.float8_e4m3)
    else:
        ct_shards = np.zeros((NC, T, NP, S), dtype=BF16)
    rdeg = np.zeros((T, NP), dtype=np.float32)
    for t in range(T):
        src = ei[t, 0].astype(np.int64)
        dst = ei[t, 1].astype(np.int64)
        cnt = np.bincount(src * NP + dst, minlength=N * NP).reshape(N, NP)
        deg = np.bincount(dst, minlength=N)
        rdeg[t, :N] = 1.0 / np.maximum(deg, 1.0)
        cnt = cnt.astype(np.float32)
        for c in range(NC):
            if merged:
                blk = np.zeros((NP, S), np.float32)
                blk[:N] = cnt[:, c * S:(c + 1) * S]
                ctc[c, t] = np.ascontiguousarray(
                    blk.reshape(KCH, 128, S).transpose(1, 0, 2)
                    .reshape(128, KCH * S)).astype(ml_dtypes.float8_e4m3)
            else:
                ct_shards[c, t, :N, :] = cnt[:, c * S:(c + 1) * S]

    embp = np.zeros((NP, H), dtype=np.float32)
    embp[:N] = emb
    xch_bf = _chunked(embp).astype(BF16)
    embT = np.ascontiguousarray(embp.T)                      # [64, NP] f32
    rb = np.ascontiguousarray(
        np.broadcast_to(rdeg[:, None, :], (T, H, NP)))       # [T, 64, NP]
    w1lT = np.ascontiguousarray(W1l.transpose(0, 2, 1))
    w1rT = np.ascontiguousarray(W1r.transpose(0, 2, 1))
    w2lT = np.ascontiguousarray(W2l.transpose(0, 2, 1))
    w2rT = np.ascontiguousarray(W2r.transpose(0, 2, 1))
    b1c = np.ascontiguousarray(b1[:, :, None])
    b2c = np.ascontiguousarray(b2[:, :, None])
    wrp = np.zeros((T, H + 1, NP), dtype=BF16)
    for t in range(T):
        wrp[t, :H, :N] = Wrec[t].T
        wrp[t, H, :N] = brec[t]

    core_ids = list(range(NC))
    trace = bool(os.environ.get("BASS_TRACE"))
    if trace:
        _ensure_ntff_hook()

    if merged:
        # single-launch path: fp8 chunk-packed CT, on-device h1 AllGather
        ncm = _build_merged()
        in_maps = [
            dict(ctc=ctc[c], xch=xch_bf,
                 xT=np.ascontiguousarray(embT[:, c * S:(c + 1) * S]),
                 rb=np.ascontiguousarray(rb[:, :, c * S:(c + 1) * S]),
                 w1l=w1lT, w1r=w1rT, b1=b1c,
                 w2l=w2lT, w2r=w2rT, b2=b2c, wrp=wrp)
            for c in core_ids
        ]
        rm = run_bass_kernel_spmd(ncm, in_maps, core_ids, trace=trace)
        h2T = np.concatenate([rm.results[c]["h2T"] for c in core_ids],
                             axis=2)
        embeddings = np.ascontiguousarray(h2T.transpose(0, 2, 1)[:, :N, :])
        reconstructions = np.empty((T, N, N), dtype=np.float32)
        for c in core_ids:
            r0 = c * S
            r1 = min(r0 + S, N)
            if r1 > r0:
                reconstructions[:, r0:r1, :] = \
                    rm.results[c]["recon"][:, :r1 - r0, :N]
        if rm.exec_time_ns is not None:
            LAST_EXEC_NS = int(rm.exec_time_ns)
            LAST_NS_PARTS = (rm.exec_time_ns,)
        return embeddings, reconstructions, emb

    # ---- launch 1: layer 1, dst-sharded over 8 cores
    nc1 = _build_layer1()
    in_maps1 = [
        dict(ct=ct_shards[c], xch=xch_bf,
             xT=np.ascontiguousarray(embT[:, c * S:(c + 1) * S]),
             rb=np.ascontiguousarray(rb[:, :, c * S:(c + 1) * S]),
             wl=w1lT, wr=w1rT, bias=b1c)
        for c in core_ids
    ]
    r1 = run_bass_kernel_spmd(nc1, in_maps1, core_ids, trace=trace,
                              tmpdir="/tmp/bass_trace_l1" if trace else None)
    res1, ns1 = r1.results, r1.exec_time_ns

    h1T = np.concatenate([res1[c]["h1T"] for c in core_ids], axis=2)
    h1 = np.ascontiguousarray(h1T.transpose(0, 2, 1))        # [T, NP, 64]
    h1[:, N:, :] = 0.0
    h1ch = np.stack([_chunked(h1[t]) for t in range(T)]).astype(BF16)

    # ---- launch 2: layer 2 + recon slab, dst-sharded over 8 cores
    nc2 = _build_layer2()
    in_maps2 = [
        dict(ct=ct_shards[c], h1ch=h1ch,
             h1T=np.ascontiguousarray(h1T[:, :, c * S:(c + 1) * S]),
             rb=np.ascontiguousarray(rb[:, :, c * S:(c + 1) * S]),
             wl=w2lT, wr=w2rT, bias=b2c, wrp=wrp)
        for c in core_ids
    ]
    r2 = run_bass_kernel_spmd(nc2, in_maps2, core_ids, trace=trace,
                              tmpdir="/tmp/bass_trace_l2" if trace else None)
    res2, ns2 = r2.results, r2.exec_time_ns

    h2T = np.concatenate([res2[c]["h2T"] for c in core_ids], axis=2)
    embeddings = np.ascontiguousarray(h2T.transpose(0, 2, 1)[:, :N, :])
    reconstructions = np.empty((T, N, N), dtype=np.float32)
    for c in core_ids:
        r0 = c * S
        r1 = min(r0 + S, N)
        if r1 > r0:
            reconstructions[:, r0:r1, :] = \
                res2[c]["recon"][:, :r1 - r0, :N]

    if ns1 is not None and ns2 is not None:
        LAST_EXEC_NS = int(ns1 + ns2)
        LAST_NS_PARTS = (ns1, ns2)
    return embeddings, reconstructions, emb


# revision 33
# speedup vs baseline: 50.3916x; 1.0113x over previous
"""Trainium2 Bass kernel for DangoPreTrain-style GNN pretraining forward.

Model (per edge type t of 3):
    h1 = relu(SAGE(emb, edges_t; W1l, b1, W1r))
    h2 = relu(SAGE(h1,  edges_t; W2l, b2, W2r))
    recon = h2 @ Wrec_t.T + brec_t            # [N, N]
Returns (embeddings [3,N,64], reconstructions [3,N,N], x_init [N,64]).

Strategy: the sparse mean-aggregation agg[i] = mean_{(j->i)} x[j] is
reformulated as a dense GEMM with the transposed edge-count matrix
CT[src, dst] = #edges(src->dst) (counts <= ~4, exact in fp8_e4m3):
aggT = x^T @ CT, then scale columns by 1/deg. CT is built on host
(bincount), chunk-packed in fp8_e4m3, and sharded over the 8 NeuronCores
by destination-node blocks (832 columns each). One SPMD launch does
everything, software-pipelined one edge type deep: each core loads its
CT shard once into SBUF and reuses it for both SAGE layers; h1 is
exchanged between cores with an on-device AllGather (layer-1 output is
PE-transposed to row layout first); the AllGather of type t overlaps
layer-1 of type t+1 and the recon slab of type t-1. Aggregation GEMMs
run bf16 x fp8 with fp32 PSUM accumulation; the 64x64 linear transforms
run in float32r; recon drains PSUM->SBUF on DVE+ACT and streams
contiguous row-stripes to HBM.
"""

import os
import sys

import numpy as np
import ml_dtypes

sys.path.insert(0, "/opt/trn_rl_repo")

import concourse.bacc as bacc
import concourse.mybir as mybir
import concourse.tile as tile
from concourse.bass_utils import run_bass_kernel_spmd


def _ensure_ntff_hook():
    """Register the NTFF profile hook missing from this image's antenv.

    trn_boot tries to install it but antenv lacks the axon_hooks module;
    shim it in-process so run_bass_kernel_spmd(trace=True) can profile.
    """
    try:
        from antenv import axon_hooks  # noqa: F401
        return True
    except ImportError:
        pass
    try:
        import types
        import antenv
        if "/root/.axon_site" not in sys.path:
            sys.path.insert(0, "/root/.axon_site")
        from trn_agent_boot.trn_boot import _ntff_profile_via_ctypes
        hook = _ntff_profile_via_ctypes("/opt/axon/libaxon_pjrt.so")
        if hook is None:
            return False
        m = types.ModuleType("antenv.axon_hooks")
        m.get_axon_ntff_profile_hook = lambda: hook
        m.set_axon_ntff_profile_hook = lambda h: None
        sys.modules["antenv.axon_hooks"] = m
        antenv.axon_hooks = m
        return True
    except Exception:
        return False


BF16 = ml_dtypes.bfloat16
F32 = mybir.dt.float32
BF = mybir.dt.bfloat16
RELU = mybir.ActivationFunctionType.Relu

N = 6607      # nodes
H = 64        # hidden
T = 3         # edge types
NP = 6656     # N padded to 52*128
KCH = NP // 128   # 52 contraction chunks
NC = 8        # cores
S = NP // NC  # 832 dst columns per core
FB = [(0, 512), (512, 320)]          # free-dim blocks of the 832-wide shard
IB = [(i * 128, 128) for i in range(6)] + [(768, 64)]  # 832-row blocks
NB = [(i * 512, 512) for i in range(13)]               # recon col blocks

LAST_EXEC_NS = None
LAST_NS_PARTS = None


def _build_merged():
    """Single-launch kernel, software-pipelined one type deep:
    L1(t) -> AllGather(t) overlaps L1(t+1); L2(t)/recon(t) run after.
    CT shard loaded once per type (fp8_e4m3, counts exact), reused by
    both layers from SBUF."""
    from concourse.masks import make_identity
    FP8 = mybir.dt.float8e4
    F32R = mybir.dt.float32r

    nc = bacc.Bacc("TRN2", target_bir_lowering=False, debug=False,
                   num_devices=NC)
    ctc = nc.dram_tensor("ctc", [T, 128, KCH * S], FP8, kind="ExternalInput")
    xch = nc.dram_tensor("xch", [128, KCH * H], BF, kind="ExternalInput")
    xT = nc.dram_tensor("xT", [H, S], F32R, kind="ExternalInput")
    rb = nc.dram_tensor("rb", [T, H, S], F32, kind="ExternalInput")
    w1l = nc.dram_tensor("w1l", [T, H, H], F32R, kind="ExternalInput")
    w1r = nc.dram_tensor("w1r", [T, H, H], F32R, kind="ExternalInput")
    b1 = nc.dram_tensor("b1", [T, H, 1], F32, kind="ExternalInput")
    w2l = nc.dram_tensor("w2l", [T, H, H], F32R, kind="ExternalInput")
    w2r = nc.dram_tensor("w2r", [T, H, H], F32R, kind="ExternalInput")
    b2 = nc.dram_tensor("b2", [T, H, 1], F32, kind="ExternalInput")
    wrp = nc.dram_tensor("wrp", [T, H + 1, NP], BF, kind="ExternalInput")
    h2T = nc.dram_tensor("h2T", [T, H, S], F32, kind="ExternalOutput")
    recon = nc.dram_tensor("recon", [T, S, NP], F32, kind="ExternalOutput")

    CA = 7 * 512
    CB = NP - CA

    with tile.TileContext(nc) as tc:
        with (
            tc.tile_pool(name="const", bufs=1) as const,
            tc.tile_pool(name="ctp", bufs=2) as ctp,
            tc.tile_pool(name="hin", bufs=1) as hin,
            tc.tile_pool(name="wrpp", bufs=2) as wrpp,
            tc.tile_pool(name="wt", bufs=2) as wt,
            tc.tile_pool(name="work", bufs=2) as work,
            tc.tile_pool(name="outp", bufs=2) as outp,
            tc.tile_pool(name="psum_a", bufs=1, space="PSUM") as psum_a,
            tc.tile_pool(name="psum_w", bufs=1, space="PSUM") as psum_w,
            tc.tile_pool(name="psum_t", bufs=1, space="PSUM") as psum_t,
            tc.tile_pool(name="psum_r", bufs=4, space="PSUM") as psum_r,
            tc.tile_pool(name="dram", bufs=3, space="DRAM") as dram,
        ):
            xch_tile = const.tile([128, KCH * H], BF)
            nc.sync.dma_start(xch_tile[:], xch[:])
            xT_tile = const.tile([H, S], F32R)
            nc.sync.dma_start(xT_tile[:], xT[:])
            ident = const.tile([H, H], BF)
            make_identity(nc, ident)

            def loads(t):
                ct_big = ctp.tile([128, KCH * S], FP8, tag="ct",
                                  name="ct_big")
                nc.sync.dma_start(ct_big[:], ctc[t])
                wrp_tile = wrpp.tile([H + 1, NP], BF, tag="wrp",
                                     name="wrp_tile")
                nc.sync.dma_start(wrp_tile[:], wrp[t])
                tl = dict(ct=ct_big, wrp=wrp_tile)
                for nm, src_ in (("rb", rb), ("w1l", w1l), ("w1r", w1r),
                                 ("w2l", w2l), ("w2r", w2r)):
                    dt_ = F32 if nm == "rb" else F32R
                    shp = [H, S] if nm == "rb" else [H, H]
                    tile_ = wt.tile(shp, dt_, tag=nm, name=nm + "_t")
                    nc.sync.dma_start(tile_[:], src_[t])
                    tl[nm] = tile_
                for nm, src_ in (("b1", b1), ("b2", b2)):
                    tile_ = wt.tile([H, 1], F32, tag=nm, name=nm + "_t")
                    nc.sync.dma_start(tile_[:], src_[t])
                    tl[nm] = tile_
                return tl

            def agg(lhs_tile, ct_big, rb_tile, wl, wr, xTr, bcol, out):
                psA = psum_a.tile([64, 512], F32, tag="psA", name="psA")
                psB = psum_a.tile([64, 320], F32, tag="psB", name="psB")
                for k in range(KCH):
                    lhs = lhs_tile[:, k * H:(k + 1) * H]
                    nc.tensor.matmul(psA[:], lhs,
                                     ct_big[:, k * S:k * S + 512],
                                     start=(k == 0), stop=(k == KCH - 1))
                    nc.tensor.matmul(psB[:], lhs,
                                     ct_big[:, k * S + 512:(k + 1) * S],
                                     start=(k == 0), stop=(k == KCH - 1))
                mm = work.tile([64, S], mybir.dt.float32r, tag="mm",
                               name="mm", bufs=1)
                nc.vector.tensor_mul(mm[:, 0:512], psA[:], rb_tile[:, 0:512])
                nc.vector.tensor_mul(mm[:, 512:S], psB[:], rb_tile[:, 512:S])
                for (f0, fs) in FB:
                    psW = psum_w.tile([64, 512], F32, name="psW")
                    nc.tensor.matmul(psW[:, :fs], wl[:], mm[:, f0:f0 + fs],
                                     start=True, stop=False)
                    nc.tensor.matmul(psW[:, :fs], wr[:], xTr[:, f0:f0 + fs],
                                     start=False, stop=True)
                    nc.scalar.activation(out[:, f0:f0 + fs], psW[:, :fs],
                                         RELU, bias=bcol[:])

            def do_l1(t, tl):
                h1T_own = work.tile([H, S], mybir.dt.float32r,
                                    tag="h1T_own", name="h1T_own")
                agg(xch_tile, tl["ct"], tl["rb"], tl["w1l"], tl["w1r"],
                    xT_tile, tl["b1"], h1T_own)
                h1Tb = work.tile([H, S], BF, tag="h1Tb", name="h1Tb",
                                 bufs=1)
                nc.vector.tensor_copy(h1Tb[:], h1T_own[:])
                gin = dram.tile([S, H], BF, tag="gin", name="gin")
                gout = dram.tile([NP, H], BF, tag="gout", name="gout",
                                 addr_space="Shared")
                for (j0, jw) in IB:
                    psT = psum_t.tile([128, H], BF, name="psT")
                    nc.tensor.transpose(psT[:jw, :], h1Tb[:, j0:j0 + jw],
                                        ident[:])
                    trs = work.tile([128, H], BF, tag="trs", name="trs")
                    nc.vector.tensor_copy(trs[:jw, :], psT[:jw, :])
                    nc.sync.dma_start(gin[j0:j0 + jw, :], trs[:jw, :])
                nc.gpsimd.collective_compute(
                    "AllGather", mybir.AluOpType.bypass,
                    replica_groups=[list(range(NC))],
                    ins=[gin[:]], outs=[gout[:]],
                )
                return h1T_own, gout

            def do_l2(t, tl, h1T_own, gout):
                h1ch = hin.tile([128, KCH * H], BF, name="h1ch")
                for k in range(KCH):
                    nc.sync.dma_start(h1ch[:, k * H:(k + 1) * H],
                                      gout[k * 128:(k + 1) * 128, :])
                hf = outp.tile([H, S], F32, tag="hf", name="hf", bufs=1)
                agg(h1ch, tl["ct"], tl["rb"], tl["w2l"], tl["w2r"],
                    h1T_own, tl["b2"], hf)
                nc.sync.dma_start(h2T[t], hf[:])
                h2b = work.tile([H + 1, S], BF, tag="h2b", name="h2b",
                                bufs=1)
                nc.vector.tensor_copy(h2b[0:H, :], hf[:])
                nc.vector.memset(h2b[H:H + 1, :], 1.0)
                return h2b

            def emit_recon(t, h2b, wrp_tile):
                for (i0, isz) in IB:
                    stA = outp.tile([128, CA], F32, tag="stA", name="stA")
                    stB = outp.tile([128, CB], F32, tag="stB", name="stB")
                    for j, (n0, nsz) in enumerate(NB):
                        psR = psum_r.tile([128, 512], F32, name="psR")
                        nc.tensor.matmul(psR[:isz, :], h2b[:, i0:i0 + isz],
                                         wrp_tile[:, n0:n0 + nsz],
                                         start=True, stop=True)
                        st = stA if n0 < CA else stB
                        o0 = n0 if n0 < CA else n0 - CA
                        if j % 2 == 0:
                            nc.vector.tensor_copy(
                                st[:isz, o0:o0 + nsz], psR[:isz, :])
                        else:
                            nc.scalar.copy(
                                st[:isz, o0:o0 + nsz], psR[:isz, :])
                    nc.sync.dma_start(recon[t, i0:i0 + isz, 0:CA],
                                      stA[:isz, :])
                    nc.sync.dma_start(recon[t, i0:i0 + isz, CA:NP],
                                      stB[:isz, :])

            pending = None   # (t, tiles, h1T_own, gout) awaiting L2+recon
            for t in range(T):
                tl = loads(t)
                h1T_own, gout = do_l1(t, tl)
                if pending is not None:
                    pt, ptl, ph1, pgout = pending
                    h2b = do_l2(pt, ptl, ph1, pgout)
                    emit_recon(pt, h2b, ptl["wrp"])
                pending = (t, tl, h1T_own, gout)
            pt, ptl, ph1, pgout = pending
            h2b = do_l2(pt, ptl, ph1, pgout)
            emit_recon(pt, h2b, ptl["wrp"])
    nc.compile()
    return nc


def _chunked(x):
    """[NP, H] row-major -> [128, KCH*H] where out[p, k*H+h] = x[k*128+p, h]."""
    return np.ascontiguousarray(
        x.reshape(KCH, 128, H).transpose(1, 0, 2).reshape(128, KCH * H))


def kernel(emb, edge_index, W1l, b1, W1r, W2l, b2, W2r, Wrec, brec):
    global LAST_EXEC_NS, LAST_NS_PARTS
    LAST_EXEC_NS = None
    LAST_NS_PARTS = None

    emb = np.asarray(emb, dtype=np.float32)
    ei = np.asarray(edge_index)
    W1l, b1, W1r = (np.asarray(a, np.float32) for a in (W1l, b1, W1r))
    W2l, b2, W2r = (np.asarray(a, np.float32) for a in (W2l, b2, W2r))
    Wrec, brec = np.asarray(Wrec, np.float32), np.asarray(brec, np.float32)

    # ---- host prep: edge-count matrices (fp8 chunk-packed), degrees,
    # padded/transposed weight views
    ctc = np.zeros((NC, T, 128, KCH * S), dtype=ml_dtypes.float8_e4m3)
    rdeg = np.zeros((T, NP), dtype=np.float32)
    for t in range(T):
        src = ei[t, 0].astype(np.int64)
        dst = ei[t, 1].astype(np.int64)
        cnt = np.bincount(src * NP + dst, minlength=N * NP).reshape(N, NP)
        deg = np.bincount(dst, minlength=N)
        rdeg[t, :N] = 1.0 / np.maximum(deg, 1.0)
        cnt = cnt.astype(np.float32)
        for c in range(NC):
            blk = np.zeros((NP, S), np.float32)
            blk[:N] = cnt[:, c * S:(c + 1) * S]
            ctc[c, t] = np.ascontiguousarray(
                blk.reshape(KCH, 128, S).transpose(1, 0, 2)
                .reshape(128, KCH * S)).astype(ml_dtypes.float8_e4m3)

    embp = np.zeros((NP, H), dtype=np.float32)
    embp[:N] = emb
    xch_bf = _chunked(embp).astype(BF16)
    embT = np.ascontiguousarray(embp.T)                      # [64, NP] f32
    rb = np.ascontiguousarray(
        np.broadcast_to(rdeg[:, None, :], (T, H, NP)))       # [T, 64, NP]
    w1lT = np.ascontiguousarray(W1l.transpose(0, 2, 1))
    w1rT = np.ascontiguousarray(W1r.transpose(0, 2, 1))
    w2lT = np.ascontiguousarray(W2l.transpose(0, 2, 1))
    w2rT = np.ascontiguousarray(W2r.transpose(0, 2, 1))
    b1c = np.ascontiguousarray(b1[:, :, None])
    b2c = np.ascontiguousarray(b2[:, :, None])
    wrp = np.zeros((T, H + 1, NP), dtype=BF16)
    for t in range(T):
        wrp[t, :H, :N] = Wrec[t].T
        wrp[t, H, :N] = brec[t]

    core_ids = list(range(NC))
    trace = bool(os.environ.get("BASS_TRACE"))
    if trace:
        _ensure_ntff_hook()

    ncm = _build_merged()
    in_maps = [
        dict(ctc=ctc[c], xch=xch_bf,
             xT=np.ascontiguousarray(embT[:, c * S:(c + 1) * S]),
             rb=np.ascontiguousarray(rb[:, :, c * S:(c + 1) * S]),
             w1l=w1lT, w1r=w1rT, b1=b1c,
             w2l=w2lT, w2r=w2rT, b2=b2c, wrp=wrp)
        for c in core_ids
    ]
    rm = run_bass_kernel_spmd(ncm, in_maps, core_ids, trace=trace)

    h2T = np.concatenate([rm.results[c]["h2T"] for c in core_ids], axis=2)
    embeddings = np.ascontiguousarray(h2T.transpose(0, 2, 1)[:, :N, :])
    reconstructions = np.empty((T, N, N), dtype=np.float32)
    for c in core_ids:
        r0 = c * S
        r1 = min(r0 + S, N)
        if r1 > r0:
            reconstructions[:, r0:r1, :] = \
                rm.results[c]["recon"][:, :r1 - r0, :N]

    if rm.exec_time_ns is not None:
        LAST_EXEC_NS = int(rm.exec_time_ns)
        LAST_NS_PARTS = (rm.exec_time_ns,)
    return embeddings, reconstructions, emb
